# revision 1
# baseline (speedup 1.0000x reference)
"""Trainium2 Bass kernel for nn_Loss_34608846471397 (center-loss style loss_fn).

Strategy: data-parallel over batch across 8 NeuronCores.  Each core gets
4096 rows of features.  Per 128-row tile:
  - indirect-DMA gather of the bf16 center row for each row's label
  - VectorE subtract, ScalarE Square+accumulate -> ||f - c_label||^2 per row
  - TensorE mask matmul accumulates per-class sums for classes C-2, C-1
    (the reference's inter-loss only uses the last class pair)
Host combines tiny per-core partials (sum of clipped distances, 2-class
sums/counts) into the two scalar losses.
"""

import os
import sys

for _p in ("/opt/trn_rl_repo", "/root/.axon_site/_ro/trn_rl_repo"):
    if os.path.isdir(_p) and _p not in sys.path:
        sys.path.insert(0, _p)

import numpy as np

import concourse.bacc as bacc
import concourse.bass as bass
import concourse.tile as tile
from concourse import mybir
from concourse.bass import IndirectOffsetOnAxis
from concourse.bass_utils import run_bass_kernel_spmd

B = 32768
D = 512
C = 1000
N_CORES = 8
BS = B // N_CORES          # rows per core
P = 128                    # partitions
NT = BS // P               # 32 row-tiles per core
CHUNK = 4                  # row-tiles per feature DMA (4*256KB = 1MB)
GAUG = D                   # gather row: one 2KB center row

_cache = {}


def _build():
    nc = bacc.Bacc("TRN2", target_bir_lowering=False, debug=False,
                   num_devices=N_CORES)
    f32 = mybir.dt.float32
    i32 = mybir.dt.int32

    feat = nc.dram_tensor("features", [BS, D], f32, kind="ExternalInput")
    lab_i = nc.dram_tensor("labels_i", [P, NT], i32, kind="ExternalInput")
    lab_f = nc.dram_tensor("labels_f", [P, NT], f32, kind="ExternalInput")
    caug = nc.dram_tensor("center_aug", [C, GAUG], mybir.dt.bfloat16,
                          kind="ExternalInput")

    intra_out = nc.dram_tensor("intra_out", [P, 1], f32, kind="ExternalOutput")
    cnt_out = nc.dram_tensor("cnt_out", [P, 2], f32, kind="ExternalOutput")
    sums_out = nc.dram_tensor("sums_out", [2, D], f32, kind="ExternalOutput")

    AF = mybir.ActivationFunctionType
    OP = mybir.AluOpType

    with tile.TileContext(nc) as tc:
        with (
            tc.tile_pool(name="feat", bufs=1) as fpool,
            tc.tile_pool(name="gath", bufs=10) as gpool,
            tc.tile_pool(name="scratch", bufs=6) as spool,
            tc.tile_pool(name="small", bufs=1) as mpool,
            tc.tile_pool(name="psum", bufs=1, space="PSUM") as ppool,
        ):
            # labels
            lab_i_sb = mpool.tile([P, NT], i32, tag="labi")
            lab_f_sb = mpool.tile([P, NT], f32, tag="labf")
            nc.sync.dma_start(out=lab_i_sb[:], in_=lab_i[:])
            nc.sync.dma_start(out=lab_f_sb[:], in_=lab_f[:])

            # masks for the two classes the inter-loss needs
            f16 = mybir.dt.float16
            mask_il = mpool.tile([P, NT, 2], f16, tag="mask")
            cnt_sb = mpool.tile([P, 2], f32, tag="cnt")
            nc.vector.tensor_scalar(out=mask_il[:, :, 0], in0=lab_f_sb[:],
                                    scalar1=float(C - 2), scalar2=None,
                                    op0=OP.is_equal)
            nc.vector.tensor_scalar(out=mask_il[:, :, 1], in0=lab_f_sb[:],
                                    scalar1=float(C - 1), scalar2=None,
                                    op0=OP.is_equal)
            nc.vector.reduce_sum(out=cnt_sb[:, 0:1], in_=mask_il[:, :, 0],
                                 axis=mybir.AxisListType.X)
            nc.vector.reduce_sum(out=cnt_sb[:, 1:2], in_=mask_il[:, :, 1],
                                 axis=mybir.AxisListType.X)

            # feature loads: 8 x 1MB chunks, tile-of-128-rows layout
            fap = feat.ap().rearrange("(n p) d -> p n d", p=P)
            f_tiles = []
            for j in range(NT // CHUNK):
                ft = fpool.tile([P, CHUNK, D], f32, tag=f"f{j}")
                nc.sync.dma_start(out=ft[:], in_=fap[:, CHUNK * j:CHUNK * (j + 1), :])
                f_tiles.append(ft)

            dist2 = mpool.tile([P, NT], f32, tag="d2")
            sums_psum = ppool.tile([2, D], f32)

            for t in range(NT):
                f_ap = f_tiles[t // CHUNK][:, t % CHUNK, :]
                g = gpool.tile([P, GAUG], mybir.dt.bfloat16, tag="g")
                nc.gpsimd.indirect_dma_start(
                    out=g[:], out_offset=None, in_=caug[:],
                    in_offset=IndirectOffsetOnAxis(ap=lab_i_sb[:, t:t + 1], axis=0),
                )
                diff = spool.tile([P, D], f32, tag="diff")
                nc.vector.tensor_tensor(out=diff[:], in0=f_ap,
                                        in1=g[:], op=OP.subtract)
                sq = spool.tile([P, D], f32, tag="sq")
                nc.scalar.activation(out=sq[:], in_=diff[:], func=AF.Square,
                                     accum_out=dist2[:, t:t + 1])
                fcast = spool.tile([P, D], f16, tag="fc")
                nc.vector.tensor_copy(out=fcast[:], in_=f_ap)
                nc.tensor.matmul(out=sums_psum[:],
                                 lhsT=mask_il[:, t, :],
                                 rhs=fcast[:],
                                 start=(t == 0), stop=(t == NT - 1))

            # epilogue
            dist = mpool.tile([P, NT], f32, tag="dist")
            nc.scalar.activation(out=dist[:], in_=dist2[:], func=AF.Sqrt)
            distc = mpool.tile([P, NT], f32, tag="distc")
            nc.vector.tensor_scalar(out=distc[:], in0=dist[:], scalar1=1e-12,
                                    scalar2=1e12, op0=OP.max, op1=OP.min)
            intra_col = mpool.tile([P, 1], f32, tag="intra")
            nc.vector.reduce_sum(out=intra_col[:], in_=distc[:],
                                 axis=mybir.AxisListType.X)
            sums_sb = mpool.tile([2, D], f32, tag="sums")
            nc.scalar.copy(out=sums_sb[:], in_=sums_psum[:])

            nc.sync.dma_start(out=intra_out[:], in_=intra_col[:])
            nc.sync.dma_start(out=cnt_out[:], in_=cnt_sb[:])
            nc.sync.dma_start(out=sums_out[:], in_=sums_sb[:])

    nc.compile()
    return nc


def _prep(features, labels, center):
    feats = np.ascontiguousarray(features, dtype=np.float32)
    labs = np.ascontiguousarray(labels, dtype=np.int32)
    cent = np.ascontiguousarray(center, dtype=np.float32)

    import ml_dtypes
    caug = cent.astype(ml_dtypes.bfloat16)

    in_maps = []
    for k in range(N_CORES):
        fs = feats[BS * k:BS * (k + 1)]
        ls = labs[BS * k:BS * (k + 1)].reshape(NT, P).T  # [P, NT]
        in_maps.append({
            "features": fs,
            "labels_i": np.ascontiguousarray(ls),
            "labels_f": np.ascontiguousarray(ls.astype(np.float32)),
            "center_aug": caug,
        })
    return in_maps


def _combine(results, labels, center):
    cent = np.asarray(center, dtype=np.float32)
    intra_sum = 0.0
    counts = np.zeros(2, dtype=np.float64)
    sums = np.zeros((2, D), dtype=np.float64)
    for r in results:
        intra_sum += float(r["intra_out"].sum(dtype=np.float64))
        counts += r["cnt_out"].sum(axis=0, dtype=np.float64)
        sums += r["sums_out"].astype(np.float64)
    intra_loss = np.float32(intra_sum / B)

    cen = np.empty((2, D), dtype=np.float32)
    for i, c in enumerate((C - 2, C - 1)):
        cnt = np.float32(max(counts[i], 1.0))
        cen[i] = (cent[c] + sums[i].astype(np.float32)) / cnt
    dvec = cen[0] - cen[1]
    d_last = np.float32(np.sqrt(np.sum(dvec * dvec, dtype=np.float32)))
    inter_loss = np.float32((2.0 / d_last) * (1.0 / (C * (C - 1))))
    return intra_loss, inter_loss


def kernel(features, labels, center, _trace=False):
    if "nc" not in _cache:
        _cache["nc"] = _build()
    nc = _cache["nc"]
    in_maps = _prep(features, labels, center)
    res = run_bass_kernel_spmd(nc, in_maps, core_ids=list(range(N_CORES)),
                               trace=_trace)
    if _trace:
        _cache["exec_time_ns"] = res.exec_time_ns
    out = _combine(res.results, labels, center)
    return out

